# revision 1
# baseline (speedup 1.0000x reference)
"""BinarizedDense TRN2 kernel: out = inputs @ (kernel > 0.5).

inputs [8192, 4096] f32, kernel [4096, 4096] f32 -> out [8192, 4096] f32.

Strategy
--------
Data parallel over tokens: each of the 8 NeuronCores computes a
[1024, 4096] output shard against the full binarized weight matrix.

Per core the matmul runs on the PE array with an exact two-term
significand split of the activations, exploiting that the binarized
weights are exactly representable at low precision:

  x = fp16(x) + lo,   lo8 = e4m3(lo * 2^9),   w8 = e4m3(B * 2^-9)
  out = fp16(x) @ B   (fp16 matmuls, 1 col/cycle)
      + lo8 @ w8      (fp8 DoubleRow matmuls: 2 K-tiles per
                       instruction at the same col/cycle rate, so the
                       lo pass costs half the hi pass)

Both B and B*2^-9 are exact in their dtypes, so every product is exact
and the only approximation is rounding lo*2^9 to e4m3 (~2^-15 |x|) plus
fp32 PSUM accumulation - measured ~5e-6 of output scale, on par with a
plain fp32 matmul's reduction-order envelope. The lo-pass matmuls
accumulate into the same PSUM group as the hi pass (the 2^9 prescale of
w8 makes them directly summable), so there is no merge pass.

Layout per core: activations staged transposed ([in_f, tok], K on
partitions), SBUF-resident (96 KB/partition); weights stream from HBM
once per output block; PSUM accumulates over K; the 8 token-tiles of a
512-wide output block occupy the 8 PSUM banks; DVE evicts PSUM->SBUF
and DMA writes the fp32 output. Activation loads are emitted just in
time inside output block 0 so the first weight tile isn't queued behind
them.
"""
from contextlib import ExitStack

import numpy as np
import ml_dtypes

import concourse.bass as bass
import concourse.tile as tile
from concourse import bacc, mybir
from concourse.bass_utils import run_bass_kernel_spmd

TOKENS, IN_F, OUT_F = 8192, 4096, 4096
N_CORES = 8
TOK = TOKENS // N_CORES      # 1024 tokens per core
P = 128                      # partitions
NT = 512                     # output free-dim tile (one PSUM bank of fp32)
KT = IN_F // P               # 32 contraction tiles
KP = KT // 2                 # 16 contraction tile pairs (DoubleRow)
MT = TOK // P                # 8 token tiles per core
NTI = OUT_F // NT            # 8 output blocks

LO_SCALE = 512.0             # 2^9: B/512 is still exact in e4m3
GRP = 2                      # k2 steps per same-mode matmul batch

_F16 = mybir.dt.float16
_F8 = mybir.dt.float8e4
_F32 = mybir.dt.float32

_cached = None


def _ensure_axon_hooks():
    """bass_utils' trace path (trace=True or BASS_TRACE=1) imports
    antenv.axon_hooks, which this image's antenv package lacks. Provide
    it, registering the ctypes NTFF hook when available so profiling
    works; with no hook registered bass_utils degrades gracefully."""
    import sys
    import types
    try:
        import antenv
        if hasattr(antenv, "axon_hooks"):
            return
        mod = types.ModuleType("antenv.axon_hooks")
        _hook = [None]
        mod.set_axon_ntff_profile_hook = lambda h: _hook.__setitem__(0, h)
        mod.get_axon_ntff_profile_hook = lambda: _hook[0]
        sys.modules["antenv.axon_hooks"] = mod
        antenv.axon_hooks = mod
        try:
            from trn_agent_boot.trn_boot import _ntff_profile_via_ctypes
            mod.set_axon_ntff_profile_hook(
                _ntff_profile_via_ctypes("/opt/axon/libaxon_pjrt.so"))
        except Exception:
            pass
    except Exception:
        pass


_ensure_axon_hooks()


def _build():
    nc = bacc.Bacc("TRN2", target_bir_lowering=False, debug=False)
    xhi = nc.dram_tensor("xhi", [IN_F, TOK], _F16, kind="ExternalInput").ap()
    xlo = nc.dram_tensor("xlo", [IN_F, TOK], _F8, kind="ExternalInput").ap()
    whi = nc.dram_tensor("whi", [IN_F, OUT_F], _F16, kind="ExternalInput").ap()
    wlo = nc.dram_tensor("wlo", [IN_F, OUT_F], _F8, kind="ExternalInput").ap()
    out = nc.dram_tensor("out", [TOK, OUT_F], _F32, kind="ExternalOutput").ap()

    with tile.TileContext(nc) as tc:
        with ExitStack() as ctx:
            xp = ctx.enter_context(tc.tile_pool(name="x", bufs=1))
            wp = ctx.enter_context(tc.tile_pool(name="w", bufs=1))
            op = ctx.enter_context(tc.tile_pool(name="o", bufs=8))
            pp = ctx.enter_context(tc.tile_pool(name="p", bufs=8, space="PSUM"))

            his = []   # KT fp16 tiles [P, TOK]
            los = []   # KP fp8 pair tiles [P, 2, TOK]

            # Warm the PE clock (HAM releases the 1.2 GHz throttle after
            # ~3.4 us of sustained activity) during the initial DMA wait,
            # so the first real matmuls run at 2.4 GHz.
            warm = wp.tile([P, NT], _F16, tag="warm", name="warm", bufs=1)
            nc.vector.memset(warm[:], 0.0)
            pwarm = pp.tile([P, NT], _F32, tag="p", name="pwarm", bufs=8)
            for i in range(24):
                nc.tensor.matmul(pwarm[:], warm[:, :P], warm[:],
                                 start=True, stop=True)

            for n in range(NTI):
                pts = [pp.tile([P, NT], _F32, tag="p", name=f"p{n}_{m}")
                       for m in range(MT)]
                nsl = slice(n * NT, (n + 1) * NT)
                for g in range(KP // GRP):
                    # Process GRP k2 steps per iteration as one fp16 batch
                    # followed by one DoubleRow batch: each plain<->DR mode
                    # boundary costs a one-time LDWEIGHTS pipeline hiccup
                    # (~163 ns, observed as periodic 379 ns matmuls), so
                    # fewer/larger same-mode batches amortize it. Weight
                    # and (block-0 only) activation loads are interleaved
                    # so the first matmul's operands land earliest.
                    k2s = [GRP * g + i for i in range(GRP)]
                    whs, wls = {}, {}
                    for k2 in k2s:
                        for j in range(2):
                            k = 2 * k2 + j
                            if n == 0 and k == 0:
                                # Shrink the first matmul's critical path:
                                # split the k=0 activation tile into the
                                # m=0 slice + the rest, so the first matmul
                                # waits on ~160 KB instead of 384 KB.
                                wh0 = wp.tile([P, NT], _F16, tag="wh",
                                              name="wh0", bufs=4 * GRP)
                                nc.sync.dma_start(wh0[:], whi[0:P, nsl])
                                xh0a = xp.tile([P, P], _F16, tag="xh0a",
                                               name="xh0a", bufs=1)
                                nc.sync.dma_start(xh0a[:], xhi[0:P, 0:P])
                                xh0b = xp.tile([P, TOK - P], _F16, tag="xh0b",
                                               name="xh0b", bufs=1)
                                nc.sync.dma_start(xh0b[:], xhi[0:P, P:TOK])
                                whs[0] = wh0
                                his.append((xh0a, xh0b))
                                continue
                            wh = wp.tile([P, NT], _F16, tag="wh",
                                         name=f"wh{n}_{k}", bufs=4 * GRP)
                            nc.sync.dma_start(
                                wh[:], whi[k * P:(k + 1) * P, nsl])
                            whs[k] = wh
                            if n == 0:
                                th = xp.tile([P, TOK], _F16, tag="xh",
                                             name=f"xh{k}", bufs=KT)
                                nc.sync.dma_start(
                                    th[:], xhi[k * P:(k + 1) * P, :])
                                his.append(th)
                        wl = wp.tile([P, 2, NT], _F8, tag="wl",
                                     name=f"wl{n}_{k2}", bufs=2 * GRP)
                        nc.sync.dma_start(
                            wl[:],
                            wlo[k2 * 2 * P:(k2 + 1) * 2 * P, nsl].rearrange(
                                "(j p) m -> p j m", p=P))
                        wls[k2] = wl
                        if n == 0:
                            tl = xp.tile([P, 2, TOK], _F8, tag="xl",
                                         name=f"xl{k2}", bufs=KP)
                            nc.sync.dma_start(
                                tl[:],
                                xlo[k2 * 2 * P:(k2 + 1) * 2 * P, :].rearrange(
                                    "(j p) m -> p j m", p=P))
                            los.append(tl)
                    # On the final group the last k2 step finishes bank m
                    # before bank m+1 (hi+DR interleaved per m) so PSUM
                    # evictions and the next block's matmuls start as early
                    # as possible; otherwise keep same-mode batches.
                    last = g == KP // GRP - 1
                    hi_k2s = k2s[:-1] if last else k2s

                    def hi_mm(m, k, wh_tile, start):
                        ms = slice(m * P, (m + 1) * P)
                        if k == 0:
                            xh0a, xh0b = his[0]
                            sta = (xh0a[:] if m == 0
                                   else xh0b[:, (m - 1) * P:m * P])
                            if isinstance(wh_tile, tuple):
                                nc.tensor.matmul(
                                    pts[m][:, :NT // 2], sta, wh_tile[0][:],
                                    start=start, stop=False)
                                nc.tensor.matmul(
                                    pts[m][:, NT // 2:], sta, wh_tile[1][:],
                                    start=start, stop=False)
                            else:
                                nc.tensor.matmul(pts[m][:], sta, wh_tile[:],
                                                 start=start, stop=False)
                        else:
                            nc.tensor.matmul(
                                pts[m][:], his[k][:, ms], wh_tile[:],
                                start=start, stop=False)

                    for k2 in hi_k2s:
                        for m in range(MT):
                            hi_mm(m, 2 * k2, whs[2 * k2], start=(k2 == 0))
                            hi_mm(m, 2 * k2 + 1, whs[2 * k2 + 1], start=False)
                    for k2 in hi_k2s:
                        for m in range(MT):
                            ms = slice(m * P, (m + 1) * P)
                            nc.tensor.matmul(
                                pts[m][:], los[k2][:, :, ms], wls[k2][:],
                                start=False, stop=False,
                                perf_mode=mybir.MatmulPerfMode.DoubleRow)
                    if last:
                        k2 = k2s[-1]
                        for m in range(MT):
                            ms = slice(m * P, (m + 1) * P)
                            nc.tensor.matmul(
                                pts[m][:], his[2 * k2][:, ms], whs[2 * k2][:],
                                start=False, stop=False)
                            nc.tensor.matmul(
                                pts[m][:], his[2 * k2 + 1][:, ms],
                                whs[2 * k2 + 1][:],
                                start=False, stop=False)
                            nc.tensor.matmul(
                                pts[m][:], los[k2][:, :, ms], wls[k2][:],
                                start=False, stop=True,
                                perf_mode=mybir.MatmulPerfMode.DoubleRow)
                for m in range(MT):
                    ot = op.tile([P, NT], _F32, tag="o", name=f"o{n}_{m}")
                    nc.vector.tensor_copy(ot[:], pts[m][:])
                    nc.sync.dma_start(out[m * P:(m + 1) * P, nsl], ot[:])
    nc.compile()
    return nc


def _get_module():
    global _cached
    if _cached is None:
        _cached = _build()
    return _cached


def _run(inputs: np.ndarray, kernel_w: np.ndarray, trace: bool = False):
    nc = _get_module()

    inputs = np.asarray(inputs, dtype=np.float32)
    kernel_w = np.asarray(kernel_w, dtype=np.float32)

    bw = kernel_w > 0.5
    whi = bw.astype(np.float16)
    wlo = (bw.astype(np.float32) / LO_SCALE).astype(ml_dtypes.float8_e4m3)
    hi = inputs.astype(np.float16)
    lo = ((inputs - hi.astype(np.float32)) * LO_SCALE).astype(
        ml_dtypes.float8_e4m3)

    in_maps = []
    for i in range(N_CORES):
        sl = slice(i * TOK, (i + 1) * TOK)
        in_maps.append({
            "xhi": np.ascontiguousarray(hi[sl].T),
            "xlo": np.ascontiguousarray(lo[sl].T),
            "whi": whi,
            "wlo": wlo,
        })

    res = None
    last_exc = None
    for attempt in range(3):
        try:
            res = run_bass_kernel_spmd(
                nc, in_maps, core_ids=list(range(N_CORES)), trace=trace)
            break
        except Exception as e:  # transient device wedges have been observed
            last_exc = e
            try:
                import jax
                jax.clear_caches()
                jax.clear_backends()
            except Exception:
                pass
    if res is None:
        raise last_exc
    full = np.concatenate([r["out"] for r in res.results], axis=0)
    return full, res


def kernel(inputs: np.ndarray, kernel: np.ndarray) -> np.ndarray:
    return _run(inputs, kernel)[0]



# revision 2
# speedup vs baseline: 1.4733x; 1.4733x over previous
"""BinarizedDense TRN2 kernel: out = inputs @ (kernel > 0.5).

inputs [8192, 4096] f32, kernel [4096, 4096] f32 -> out [8192, 4096] f32.

Strategy
--------
Data parallel over tokens: each of the 8 NeuronCores computes a
[1024, 4096] output shard against the full binarized weight matrix.

Single fp16 pass on the PE array. The binarized weights {0,1} are exact
in fp16; the only approximation is rounding the activations to fp16
(~2^-11 relative), which lands the output at ~3e-4 relative error --
far inside the 2e-2 gate -- at 1 column/cycle. (TRN2/cayman has no
uint8 matmul and DoubleRow is fp8-only at 2x, so any scheme with a
correction pass costs the same 1.0 fp16-equivalents; a lone fp8 pass
would be 2x faster but ~3.6e-2 error. 1.0 is the accuracy/ISA floor.)

Layout per core: activations staged transposed ([in_f, tok], K on
partitions), SBUF-resident (32 tiles of [128, 1024] fp16 = 8 MB);
weights stream from HBM once per 512-wide output block from a
host-pretiled contiguous layout [n_blk, k_tile, 128, 512]; PSUM
accumulates over K with the 8 token-tiles of a block in the 8 PSUM
banks; DVE evicts PSUM->SBUF and DMA writes the fp32 output.
Activation loads are emitted just in time inside output block 0 so the
first weight tile isn't queued behind them.
"""
from contextlib import ExitStack

import numpy as np

import concourse.bass as bass
import concourse.tile as tile
from concourse import bacc, mybir
from concourse.bass_utils import run_bass_kernel_spmd

TOKENS, IN_F, OUT_F = 8192, 4096, 4096
N_CORES = 8
TOK = TOKENS // N_CORES      # 1024 tokens per core
P = 128                      # partitions
NT = 512                     # output free-dim tile (one PSUM bank of fp32)
KT = IN_F // P               # 32 contraction tiles
MT = TOK // P                # 8 token tiles per core
NTI = OUT_F // NT            # 8 output blocks

_F16 = mybir.dt.float16
_F32 = mybir.dt.float32

_cached = None


def _ensure_axon_hooks():
    """bass_utils' trace path (trace=True or BASS_TRACE=1) imports
    antenv.axon_hooks, which this image's antenv package lacks. Provide
    it, registering the ctypes NTFF hook when available so profiling
    works; with no hook registered bass_utils degrades gracefully."""
    import sys
    import types
    try:
        import antenv
        if hasattr(antenv, "axon_hooks"):
            return
        mod = types.ModuleType("antenv.axon_hooks")
        _hook = [None]
        mod.set_axon_ntff_profile_hook = lambda h: _hook.__setitem__(0, h)
        mod.get_axon_ntff_profile_hook = lambda: _hook[0]
        sys.modules["antenv.axon_hooks"] = mod
        antenv.axon_hooks = mod
        try:
            from trn_agent_boot.trn_boot import _ntff_profile_via_ctypes
            mod.set_axon_ntff_profile_hook(
                _ntff_profile_via_ctypes("/opt/axon/libaxon_pjrt.so"))
        except Exception:
            pass
    except Exception:
        pass


_ensure_axon_hooks()


def _build():
    nc = bacc.Bacc("TRN2", target_bir_lowering=False, debug=False)
    xhi = nc.dram_tensor("xhi", [IN_F, TOK], _F16, kind="ExternalInput").ap()
    # host-pretiled contiguous weight blocks: [NTI, KT, P, NT]
    whi = nc.dram_tensor("whi", [NTI, KT, P, NT], _F16,
                         kind="ExternalInput").ap()
    out = nc.dram_tensor("out", [TOK, OUT_F], _F32, kind="ExternalOutput").ap()

    with tile.TileContext(nc) as tc:
        with ExitStack() as ctx:
            xp = ctx.enter_context(tc.tile_pool(name="x", bufs=1))
            wp = ctx.enter_context(tc.tile_pool(name="w", bufs=1))
            op = ctx.enter_context(tc.tile_pool(name="o", bufs=8))
            pp = ctx.enter_context(tc.tile_pool(name="p", bufs=8, space="PSUM"))

            his = []   # KT fp16 tiles [P, TOK]

            # Warm the PE clock (HAM releases the 1.2 GHz throttle after
            # ~3.4 us of sustained activity) during the initial DMA wait,
            # so the first real matmuls run at 2.4 GHz.
            warm = wp.tile([P, NT], _F16, tag="warm", name="warm", bufs=1)
            nc.vector.memset(warm[:], 0.0)
            pwarm = pp.tile([P, NT], _F32, tag="p", name="pwarm", bufs=8)
            for i in range(24):
                nc.tensor.matmul(pwarm[:], warm[:, :P], warm[:],
                                 start=True, stop=True)

            for n in range(NTI):
                pts = [pp.tile([P, NT], _F32, tag="p", name=f"p{n}_{m}")
                       for m in range(MT)]
                for k in range(KT):
                    if n == 0 and k == 0:
                        # Shrink the first matmul's critical path: split
                        # the k=0 activation tile into the m=0 slice +
                        # the rest, so the first matmul waits on ~160 KB
                        # instead of 384 KB.
                        wh = wp.tile([P, NT], _F16, tag="wh",
                                     name="wh0", bufs=8)
                        nc.sync.dma_start(wh[:], whi[0, 0])
                        xh0a = xp.tile([P, P], _F16, tag="xh0a",
                                       name="xh0a", bufs=1)
                        nc.sync.dma_start(xh0a[:], xhi[0:P, 0:P])
                        xh0b = xp.tile([P, TOK - P], _F16, tag="xh0b",
                                       name="xh0b", bufs=1)
                        nc.sync.dma_start(xh0b[:], xhi[0:P, P:TOK])
                        his.append((xh0a, xh0b))
                    else:
                        wh = wp.tile([P, NT], _F16, tag="wh",
                                     name=f"wh{n}_{k}", bufs=8)
                        nc.sync.dma_start(wh[:], whi[n, k])
                        if n == 0:
                            th = xp.tile([P, TOK], _F16, tag="xh",
                                         name=f"xh{k}", bufs=KT)
                            nc.sync.dma_start(
                                th[:], xhi[k * P:(k + 1) * P, :])
                            his.append(th)
                    for m in range(MT):
                        ms = slice(m * P, (m + 1) * P)
                        if k == 0:
                            xh0a, xh0b = his[0]
                            sta = (xh0a[:] if m == 0
                                   else xh0b[:, (m - 1) * P:m * P])
                        else:
                            sta = his[k][:, ms]
                        nc.tensor.matmul(pts[m][:], sta, wh[:],
                                         start=(k == 0), stop=(k == KT - 1))
                nsl = slice(n * NT, (n + 1) * NT)
                for m in range(MT):
                    ot = op.tile([P, NT], _F32, tag="o", name=f"o{n}_{m}")
                    nc.vector.tensor_copy(ot[:], pts[m][:])
                    nc.sync.dma_start(out[m * P:(m + 1) * P, nsl], ot[:])
    nc.compile()
    return nc


def _get_module():
    global _cached
    if _cached is None:
        _cached = _build()
    return _cached


def _prep_host(inputs: np.ndarray, kernel_w: np.ndarray):
    inputs = np.asarray(inputs, dtype=np.float32)
    kernel_w = np.asarray(kernel_w, dtype=np.float32)

    whi = (kernel_w > 0.5).astype(np.float16)
    # pretile to [NTI, KT, P, NT] so each weight-tile DMA is one
    # contiguous 128 KB read
    whi = np.ascontiguousarray(
        whi.reshape(KT, P, NTI, NT).transpose(2, 0, 1, 3))
    hi = inputs.astype(np.float16)
    return hi, whi


def _run(inputs: np.ndarray, kernel_w: np.ndarray, trace: bool = False):
    nc = _get_module()
    hi, whi = _prep_host(inputs, kernel_w)

    in_maps = []
    for i in range(N_CORES):
        sl = slice(i * TOK, (i + 1) * TOK)
        in_maps.append({
            "xhi": np.ascontiguousarray(hi[sl].T),
            "whi": whi,
        })

    res = None
    last_exc = None
    for attempt in range(3):
        try:
            res = run_bass_kernel_spmd(
                nc, in_maps, core_ids=list(range(N_CORES)), trace=trace)
            break
        except Exception as e:  # transient device wedges have been observed
            last_exc = e
            try:
                import jax
                jax.clear_caches()
                jax.clear_backends()
            except Exception:
                pass
    if res is None:
        raise last_exc
    full = np.concatenate([r["out"] for r in res.results], axis=0)
    return full, res


def kernel(inputs: np.ndarray, kernel: np.ndarray) -> np.ndarray:
    return _run(inputs, kernel)[0]


# revision 5
# speedup vs baseline: 1.4901x; 1.0115x over previous
"""BinarizedDense TRN2 kernel: out = inputs @ (kernel > 0.5).

inputs [8192, 4096] f32, kernel [4096, 4096] f32 -> out [8192, 4096] f32.

Strategy
--------
Data parallel over tokens: each of the 8 NeuronCores computes a
[1024, 4096] output shard against the full binarized weight matrix.

Single fp16 pass on the PE array. The binarized weights {0,1} are exact
in fp16; the only approximation is rounding the activations to fp16
(~2^-11 relative), which lands the output at ~3e-4 relative error --
far inside the 2e-2 gate -- at 1 column/cycle. (TRN2/cayman has no
uint8 matmul and DoubleRow is fp8-only at 2x, so any scheme with a
correction pass costs the same 1.0 fp16-equivalents; a lone fp8 pass
would be 2x faster but ~3.6e-2 error. 1.0 is the accuracy/ISA floor.)

Layout per core: activations staged transposed ([in_f, tok], K on
partitions), SBUF-resident (32 tiles of [128, 1024] fp16 = 8 MB);
weights stream from HBM once per 512-wide output block from a
host-pretiled contiguous layout [n_blk, k_tile, 128, 512]; PSUM
accumulates over K with the 8 token-tiles of a block in the 8 PSUM
banks; DVE evicts PSUM->SBUF and DMA writes the fp32 output.
Activation loads are emitted just in time inside output block 0 so the
first weight tile isn't queued behind them.
"""
from contextlib import ExitStack

import numpy as np

import concourse.bass as bass
import concourse.tile as tile
from concourse import bacc, mybir
from concourse.bass_utils import run_bass_kernel_spmd

TOKENS, IN_F, OUT_F = 8192, 4096, 4096
N_CORES = 8
TOK = TOKENS // N_CORES      # 1024 tokens per core
P = 128                      # partitions
NT = 512                     # output free-dim tile (one PSUM bank of fp32)
KT = IN_F // P               # 32 contraction tiles
MT = TOK // P                # 8 token tiles per core
NTI = OUT_F // NT            # 8 output blocks

_F16 = mybir.dt.float16
_F32 = mybir.dt.float32

_cached = None


def _ensure_axon_hooks():
    """bass_utils' trace path (trace=True or BASS_TRACE=1) imports
    antenv.axon_hooks, which this image's antenv package lacks. Provide
    it, registering the ctypes NTFF hook when available so profiling
    works; with no hook registered bass_utils degrades gracefully."""
    import sys
    import types
    try:
        import antenv
        if hasattr(antenv, "axon_hooks"):
            return
        mod = types.ModuleType("antenv.axon_hooks")
        _hook = [None]
        mod.set_axon_ntff_profile_hook = lambda h: _hook.__setitem__(0, h)
        mod.get_axon_ntff_profile_hook = lambda: _hook[0]
        sys.modules["antenv.axon_hooks"] = mod
        antenv.axon_hooks = mod
        try:
            from trn_agent_boot.trn_boot import _ntff_profile_via_ctypes
            mod.set_axon_ntff_profile_hook(
                _ntff_profile_via_ctypes("/opt/axon/libaxon_pjrt.so"))
        except Exception:
            pass
    except Exception:
        pass


_ensure_axon_hooks()


def _build():
    nc = bacc.Bacc("TRN2", target_bir_lowering=False, debug=False)
    xhi = nc.dram_tensor("xhi", [IN_F, TOK], _F16, kind="ExternalInput").ap()
    # host-pretiled contiguous weight blocks: [NTI, KT, P, NT]
    whi = nc.dram_tensor("whi", [NTI, KT, P, NT], _F16,
                         kind="ExternalInput").ap()
    out = nc.dram_tensor("out", [TOK, OUT_F], _F32, kind="ExternalOutput").ap()

    with tile.TileContext(nc) as tc:
        with ExitStack() as ctx:
            xp = ctx.enter_context(tc.tile_pool(name="x", bufs=1))
            wp = ctx.enter_context(tc.tile_pool(name="w", bufs=1))
            op = ctx.enter_context(tc.tile_pool(name="o", bufs=8))
            pp = ctx.enter_context(tc.tile_pool(name="p", bufs=8, space="PSUM"))

            his = []   # KT fp16 tiles [P, TOK]
            whs = {}   # weight tiles of the current block

            def load_tiles(n):
                for k in range(KT):
                    if n == 0 and k == 0:
                        # Shrink the first matmul's critical path: split
                        # the k=0 activation tile into the m=0 slice +
                        # the rest, so the first matmul waits on ~160 KB
                        # instead of 384 KB.
                        wh = wp.tile([P, NT], _F16, tag="wh",
                                     name="wh0", bufs=WB)
                        nc.sync.dma_start(wh[:], whi[0, 0])
                        xh0a = xp.tile([P, P], _F16, tag="xh0a",
                                       name="xh0a", bufs=1)
                        nc.sync.dma_start(xh0a[:], xhi[0:P, 0:P])
                        xh0b = xp.tile([P, TOK - P], _F16, tag="xh0b",
                                       name="xh0b", bufs=1)
                        nc.sync.dma_start(xh0b[:], xhi[0:P, P:TOK])
                        his.append((xh0a, xh0b))
                    else:
                        wh = wp.tile([P, NT], _F16, tag="wh",
                                     name=f"wh{n}_{k}", bufs=WB)
                        nc.sync.dma_start(wh[:], whi[n, k])
                        if n == 0:
                            th = xp.tile([P, TOK], _F16, tag="xh",
                                         name=f"xh{k}", bufs=KT)
                            nc.sync.dma_start(
                                th[:], xhi[k * P:(k + 1) * P, :])
                            his.append(th)
                    yield k, wh

            def sta_ap(k, m):
                if k == 0:
                    xh0a, xh0b = his[0]
                    return (xh0a[:] if m == 0
                            else xh0b[:, (m - 1) * P:m * P])
                return his[k][:, m * P:(m + 1) * P]

            def evict(n, m, pt):
                ot = op.tile([P, NT], _F32, tag="o", name=f"o{n}_{m}")
                nc.vector.tensor_copy(ot[:], pt[:])
                nc.sync.dma_start(
                    out[m * P:(m + 1) * P, n * NT:(n + 1) * NT], ot[:])

            WB = KT   # full-block weight ring: deep prefetch, one slot/tile
            KH = KT // 2
            for n in range(NTI):
                pts = [pp.tile([P, NT], _F32, tag="p", name=f"p{n}_{m}")
                       for m in range(MT)]
                last = n == NTI - 1
                if last:
                    # Last block: run the second half of K in m-outer
                    # order so PSUM banks retire one at a time and
                    # evictions + output DMA overlap the remaining
                    # matmuls instead of forming a serial tail.
                    tiles = dict(load_tiles(n))
                    for k in range(KH):
                        for m in range(MT):
                            nc.tensor.matmul(pts[m][:], sta_ap(k, m),
                                             tiles[k][:],
                                             start=(k == 0), stop=False)
                    for m in range(MT):
                        for k in range(KH, KT):
                            nc.tensor.matmul(pts[m][:], sta_ap(k, m),
                                             tiles[k][:],
                                             start=False, stop=(k == KT - 1))
                        evict(n, m, pts[m])
                else:
                    for k, wh in load_tiles(n):
                        for m in range(MT):
                            nc.tensor.matmul(pts[m][:], sta_ap(k, m), wh[:],
                                             start=(k == 0),
                                             stop=(k == KT - 1))
                    for m in range(MT):
                        evict(n, m, pts[m])
    nc.compile()
    return nc


def _get_module():
    global _cached
    if _cached is None:
        _cached = _build()
    return _cached


def _prep_host(inputs: np.ndarray, kernel_w: np.ndarray):
    inputs = np.asarray(inputs, dtype=np.float32)
    kernel_w = np.asarray(kernel_w, dtype=np.float32)

    whi = (kernel_w > 0.5).astype(np.float16)
    # pretile to [NTI, KT, P, NT] so each weight-tile DMA is one
    # contiguous 128 KB read
    whi = np.ascontiguousarray(
        whi.reshape(KT, P, NTI, NT).transpose(2, 0, 1, 3))
    hi = inputs.astype(np.float16)
    return hi, whi


def _run(inputs: np.ndarray, kernel_w: np.ndarray, trace: bool = False):
    nc = _get_module()
    hi, whi = _prep_host(inputs, kernel_w)

    in_maps = []
    for i in range(N_CORES):
        sl = slice(i * TOK, (i + 1) * TOK)
        in_maps.append({
            "xhi": np.ascontiguousarray(hi[sl].T),
            "whi": whi,
        })

    res = None
    last_exc = None
    for attempt in range(3):
        try:
            res = run_bass_kernel_spmd(
                nc, in_maps, core_ids=list(range(N_CORES)), trace=trace)
            break
        except Exception as e:  # transient device wedges have been observed
            last_exc = e
            try:
                import jax
                jax.clear_caches()
                jax.clear_backends()
            except Exception:
                pass
    if res is None:
        raise last_exc
    full = np.concatenate([r["out"] for r in res.results], axis=0)
    return full, res


def kernel(inputs: np.ndarray, kernel: np.ndarray) -> np.ndarray:
    return _run(inputs, kernel)[0]
